# revision 4
# baseline (speedup 1.0000x reference)
"""Trainium2 Bass kernel for a 2-layer GAT (B=8, N=1024, F=256, D=64, H=8, C=256).

Sharding: data-parallel over batch — one batch element per NeuronCore (8 cores).

Per-core algorithm (all layouts chosen so softmax needs no transposes):
  h      = x @ W_all (+Wb)                          [n, 512]   PE, bf16
  sl/sr  = x @ V_l / V_r (+consts, ab folded)       [n, 16]    PE (same lhsT)
  scoresT[j,i] = LR(sl_i + sr_j + ab) + mask        [j, i]     built directly
      S1: x = (sl_bcast + sr_j) + logmT   one scalar_tensor_tensor per j-chunk
      S2: u = (x*0.2) max x               one scalar_tensor_tensor (LeakyReLU)
      S3: e = Exp(u)                      one ACT sweep
      (mask folded additively pre-LR as -16384; exp underflows to exact 0)
  agg:   out[i, 65h] = sum_j e[j,i] * [h_h | 1]     PE; ones col gives Z_i
  hh     = num / Z ; z = ELU(hh) = relu(hh) + min(exp(hh)-1, 0)
  layer 2 identical with g = z @ Wo (+u_l/u_r cols for tl/tr), C=256
  out    = ELU(a2 @ g / Z2) + x
"""

import numpy as np
import ml_dtypes
from contextlib import ExitStack

BF16 = ml_dtypes.bfloat16
B, N, F, D, H, C = 8, 1024, 256, 64, 8, 256
HD = H * D  # 512
NEGM = -16384.0  # mask offset; LR then exp underflows to exact 0
ALPHA = 0.2

_CACHE = {}


def _build_program():
    import concourse.bacc as bacc
    import concourse.bass as bass
    import concourse.mybir as mybir
    from concourse.tile import TileContext
    from concourse.masks import make_identity

    dt = mybir.dt
    Alu = mybir.AluOpType
    Act = mybir.ActivationFunctionType

    nc = bacc.Bacc()

    xt = nc.declare_dram_parameter("xt", [F + 1, N], dt.bfloat16, isOutput=False)
    xs = nc.declare_dram_parameter("xs", [N, F], dt.float32, isOutput=False)
    lmt = nc.declare_dram_parameter("lmt", [N, N], dt.bfloat16, isOutput=False)
    wp = nc.declare_dram_parameter("wp", [F + 1, HD], dt.bfloat16, isOutput=False)
    vlr = nc.declare_dram_parameter("vlr", [F + 1, 2 * H], dt.bfloat16, isOutput=False)
    wo = nc.declare_dram_parameter("wo", [HD + 1, C + 2], dt.bfloat16, isOutput=False)
    out_d = nc.declare_dram_parameter("out", [N, C], dt.float32, isOutput=True)

    rows_d = nc.dram_tensor("rows_bounce", [2 * H + 2, N], dt.bfloat16)

    NCH = N // 128  # 8 chunks of 128 nodes

    def bcast128(row_ap):
        # [1, N] DRAM row -> [128, N] partition-broadcast read for DMA
        return bass.AP(
            tensor=row_ap.tensor,
            offset=row_ap.offset,
            ap=[[0, 128]] + list(row_ap.ap),
        )

    with TileContext(nc) as tc:
        with ExitStack() as ctx:
            cons = ctx.enter_context(tc.tile_pool(name="cons", bufs=1))
            bc = ctx.enter_context(tc.tile_pool(name="bc", bufs=3))
            eb = ctx.enter_context(tc.tile_pool(name="eb", bufs=5))
            wk = ctx.enter_context(tc.tile_pool(name="wk", bufs=3))
            sm = ctx.enter_context(tc.tile_pool(name="sm", bufs=4))
            pmm = ctx.enter_context(tc.tile_pool(name="pmm", bufs=3, space="PSUM"))
            pm2 = ctx.enter_context(tc.tile_pool(name="pm2", bufs=2, space="PSUM"))
            ptp = ctx.enter_context(tc.tile_pool(name="ptp", bufs=1, space="PSUM"))

            # ---------- constants / params ----------
            ident_f = cons.tile([128, 128], dt.float32)
            make_identity(nc, ident_f[:, :])
            ident_b = cons.tile([128, 128], dt.bfloat16)
            make_identity(nc, ident_b[:, :])

            xt_sb = cons.tile([128, 2 * N], dt.bfloat16)
            nc.sync.dma_start(out=xt_sb[:, 0:N], in_=xt[0:128, :])
            nc.sync.dma_start(out=xt_sb[:, N : 2 * N], in_=xt[128:256, :])
            xt_one = cons.tile([1, N], dt.bfloat16)
            nc.sync.dma_start(out=xt_one[:, :], in_=xt[256:257, :])

            wp_sb = cons.tile([128, 2 * HD], dt.bfloat16)
            nc.sync.dma_start(out=wp_sb[:, 0:HD], in_=wp[0:128, :])
            nc.sync.dma_start(out=wp_sb[:, HD : 2 * HD], in_=wp[128:256, :])
            wp_one = cons.tile([1, HD], dt.bfloat16)
            nc.sync.dma_start(out=wp_one[:, :], in_=wp[256:257, :])

            vlr_sb = cons.tile([128, 4 * H], dt.bfloat16)
            nc.sync.dma_start(out=vlr_sb[:, 0 : 2 * H], in_=vlr[0:128, :])
            nc.sync.dma_start(out=vlr_sb[:, 2 * H : 4 * H], in_=vlr[128:256, :])
            vlr_one = cons.tile([1, 2 * H], dt.bfloat16)
            nc.sync.dma_start(out=vlr_one[:, :], in_=vlr[256:257, :])

            wo_sb = cons.tile([128, 4 * (C + 2)], dt.bfloat16)
            for k in range(4):
                nc.sync.dma_start(
                    out=wo_sb[:, k * (C + 2) : (k + 1) * (C + 2)],
                    in_=wo[k * 128 : (k + 1) * 128, :],
                )
            wo_one = cons.tile([1, C + 2], dt.bfloat16)
            nc.sync.dma_start(out=wo_one[:, :], in_=wo[HD : HD + 1, :])

            lmt_sb = cons.tile([128, NCH * N], dt.bfloat16)
            for c in range(NCH):
                nc.sync.dma_start(
                    out=lmt_sb[:, c * N : (c + 1) * N],
                    in_=lmt[c * 128 : (c + 1) * 128, :],
                )

            xs_sb = cons.tile([128, NCH * F], dt.float32)
            for c in range(NCH):
                nc.sync.dma_start(
                    out=xs_sb[:, c * F : (c + 1) * F],
                    in_=xs[c * 128 : (c + 1) * 128, :],
                )

            # ---------- phase 1: h = x@W_all, sl/sr = x@VLR ----------
            hx = cons.tile([128, NCH * 8 * 66], dt.bfloat16)  # [h(64)|1|pad] per head
            nc.vector.memset(hx[:, :], 1.0)
            slsr = cons.tile([128, NCH * 16], dt.float32)

            for n in range(NCH):
                ph = pmm.tile([128, HD], dt.float32, tag="mm")
                ps = pm2.tile([128, 16], dt.float32, tag="mm2")
                for k in range(2):
                    lt = xt_sb[:, k * N + n * 128 : k * N + n * 128 + 128]
                    nc.tensor.matmul(
                        ph[:, :], lt, wp_sb[:, k * HD : (k + 1) * HD],
                        start=(k == 0), stop=False,
                    )
                    nc.tensor.matmul(
                        ps[:, :], lt, vlr_sb[:, k * 16 : (k + 1) * 16],
                        start=(k == 0), stop=False,
                    )
                lt1 = xt_one[:, n * 128 : n * 128 + 128]
                nc.tensor.matmul(ph[:, :], lt1, wp_one[:, :], start=False, stop=True)
                nc.tensor.matmul(ps[:, :], lt1, vlr_one[:, :], start=False, stop=True)

                # exit h -> hx (bf16, 66-stride blocks; ones cols pre-set)
                hx_v = hx[:, n * 528 : (n + 1) * 528].rearrange(
                    "p (h s) -> p h s", s=66
                )[:, :, 0:64]
                ph_v = ph[:, :].rearrange("p (h s) -> p h s", s=64)
                nc.scalar.activation(hx_v, ph_v, Act.Copy)
                nc.vector.tensor_copy(out=slsr[:, n * 16 : (n + 1) * 16], in_=ps[:, :])

            # ---------- phase 1b: transpose sl/sr to rows; bounce via DRAM ----
            pt = ptp.tile([16, N], dt.float32, tag="tp")
            for n in range(NCH):
                nc.tensor.transpose(
                    pt[:, n * 128 : (n + 1) * 128],
                    slsr[:, n * 16 : (n + 1) * 16],
                    ident_f[:, :],
                )
            slsrT = cons.tile([16, N], dt.bfloat16)
            nc.scalar.activation(slsrT[:, :], pt[:, :], Act.Copy)
            nc.sync.dma_start(out=rows_d[0:16, :], in_=slsrT[:, :])

            # ---------- phase 2: attention per head, quads for agg exits ------
            zacc = cons.tile([128, NCH * 528], dt.float32)
            z_sb = cons.tile([128, NCH * HD], dt.bfloat16)

            for q in range(2):
                ebufs = []
                for hq in range(4):
                    h = 4 * q + hq
                    slb = bc.tile([128, N], dt.bfloat16, tag="slb")
                    nc.sync.dma_start(out=slb[:, :], in_=bcast128(rows_d[h : h + 1, :]))
                    e = eb.tile([128, NCH * N], dt.bfloat16, tag="e")
                    for c in range(NCH):
                        # S1: (sl_bcast + sr_j) + logmT
                        nc.vector.scalar_tensor_tensor(
                            out=e[:, c * N : (c + 1) * N],
                            in0=slb[:, :],
                            scalar=slsr[:, c * 16 + 8 + h : c * 16 + 9 + h],
                            in1=lmt_sb[:, c * N : (c + 1) * N],
                            op0=Alu.add,
                            op1=Alu.add,
                        )
                    for half in range(2):
                        s = half * (NCH * N // 2)
                        t = s + NCH * N // 2
                        # S2: LeakyReLU in place: (x*0.2) max x
                        nc.vector.scalar_tensor_tensor(
                            out=e[:, s:t], in0=e[:, s:t], scalar=ALPHA,
                            in1=e[:, s:t], op0=Alu.mult, op1=Alu.max,
                        )
                    # S3: exp in place
                    nc.scalar.activation(e[:, :], e[:, :], Act.Exp)
                    ebufs.append(e)

                for ic in range(NCH):
                    pa = pmm.tile([128, 260], dt.float32, tag="mm")
                    for hq in range(4):
                        h = 4 * q + hq
                        e = ebufs[hq]
                        for jc in range(NCH):
                            nc.tensor.matmul(
                                pa[:, hq * 65 : hq * 65 + 65],
                                e[:, jc * N + ic * 128 : jc * N + ic * 128 + 128],
                                hx[:, jc * 528 + h * 66 : jc * 528 + h * 66 + 65],
                                start=(jc == 0),
                                stop=(jc == NCH - 1),
                            )
                    nc.scalar.activation(
                        zacc[:, ic * 528 + q * 264 : ic * 528 + q * 264 + 260],
                        pa[:, :],
                        Act.Copy,
                    )

            # ---------- phase 2b: divide by Z, ELU -> z ----------
            rz_all = cons.tile([128, NCH * 8], dt.float32)
            for ic in range(NCH):
                rzrep = wk.tile([128, HD], dt.float32, tag="rzrep")
                hh = wk.tile([128, HD], dt.bfloat16, tag="hh")
                for q in range(2):
                    zq = zacc[:, ic * 528 + q * 264 : ic * 528 + q * 264 + 260]
                    zq_v = zq.rearrange("p (h s) -> p h s", s=65)
                    rz = rz_all[:, ic * 8 + q * 4 : ic * 8 + q * 4 + 4]
                    nc.vector.reciprocal(
                        rz[:, :].rearrange("p (h s) -> p h s", s=1),
                        zq_v[:, :, 64:65],
                    )
                    nc.vector.tensor_copy(
                        out=rzrep[:, q * 256 : (q + 1) * 256].rearrange(
                            "p (h s) -> p h s", s=64
                        ),
                        in_=rz[:, :]
                        .rearrange("p (h s) -> p h s", s=1)
                        .to_broadcast([128, 4, 64]),
                    )
                    nc.vector.tensor_tensor(
                        out=hh[:, q * 256 : (q + 1) * 256].rearrange(
                            "p (h s) -> p h s", s=64
                        ),
                        in0=zq_v[:, :, 0:64],
                        in1=rzrep[:, q * 256 : (q + 1) * 256].rearrange(
                            "p (h s) -> p h s", s=64
                        ),
                        op=Alu.mult,
                    )
                ee = wk.tile([128, HD], dt.bfloat16, tag="ee")
                nc.scalar.activation(ee[:, :], hh[:, :], Act.Exp)
                r1 = wk.tile([128, HD], dt.bfloat16, tag="r1")
                nc.vector.tensor_scalar(
                    out=r1[:, :], in0=ee[:, :], scalar1=-1.0, scalar2=0.0,
                    op0=Alu.add, op1=Alu.min,
                )
                nc.vector.scalar_tensor_tensor(
                    out=z_sb[:, ic * HD : (ic + 1) * HD],
                    in0=hh[:, :], scalar=0.0, in1=r1[:, :],
                    op0=Alu.max, op1=Alu.add,
                )

            # ---------- phase 3: zT via PE transpose ----------
            zt_sb = cons.tile([128, 4 * N], dt.bfloat16)
            zt_one = cons.tile([1, N], dt.bfloat16)
            nc.vector.memset(zt_one[:, :], 1.0)
            for kc in range(4):
                pz = pmm.tile([128, N], dt.bfloat16, tag="mm")
                for ic in range(NCH):
                    nc.tensor.transpose(
                        pz[:, ic * 128 : (ic + 1) * 128],
                        z_sb[:, ic * HD + kc * 128 : ic * HD + kc * 128 + 128],
                        ident_b[:, :],
                    )
                nc.scalar.activation(zt_sb[:, kc * N : (kc + 1) * N], pz[:, :], Act.Copy)

            # ---------- phase 4: g = z@Wo (+tl/tr cols) ----------
            gx = cons.tile([128, NCH * 260], dt.bfloat16)
            nc.vector.memset(gx[:, :], 1.0)
            glgr = cons.tile([128, NCH * 2], dt.float32)
            for n in range(NCH):
                pg = pmm.tile([128, C + 2], dt.float32, tag="mm")
                for kc in range(4):
                    nc.tensor.matmul(
                        pg[:, :],
                        zt_sb[:, kc * N + n * 128 : kc * N + n * 128 + 128],
                        wo_sb[:, kc * (C + 2) : (kc + 1) * (C + 2)],
                        start=(kc == 0), stop=False,
                    )
                nc.tensor.matmul(
                    pg[:, :], zt_one[:, n * 128 : n * 128 + 128], wo_one[:, :],
                    start=False, stop=True,
                )
                nc.scalar.activation(
                    gx[:, n * 260 : n * 260 + C], pg[:, 0:C], Act.Copy
                )
                nc.vector.tensor_copy(
                    out=glgr[:, n * 2 : (n + 1) * 2], in_=pg[:, C : C + 2]
                )

            # ---------- phase 4b: gl/gr rows ----------
            pt2 = ptp.tile([2, N], dt.float32, tag="tp")
            for n in range(NCH):
                nc.tensor.transpose(
                    pt2[:, n * 128 : (n + 1) * 128],
                    glgr[:, n * 2 : (n + 1) * 2],
                    ident_f[:, :],
                )
            ggT = cons.tile([2, N], dt.bfloat16)
            nc.scalar.activation(ggT[:, :], pt2[:, :], Act.Copy)
            nc.sync.dma_start(out=rows_d[16:18, :], in_=ggT[:, :])

            # ---------- phase 5: output attention layer ----------
            glb = bc.tile([128, N], dt.bfloat16, tag="slb")
            nc.sync.dma_start(out=glb[:, :], in_=bcast128(rows_d[16:17, :]))
            e2 = eb.tile([128, NCH * N], dt.bfloat16, tag="e")
            for c in range(NCH):
                nc.vector.scalar_tensor_tensor(
                    out=e2[:, c * N : (c + 1) * N],
                    in0=glb[:, :],
                    scalar=glgr[:, c * 2 + 1 : c * 2 + 2],
                    in1=lmt_sb[:, c * N : (c + 1) * N],
                    op0=Alu.add, op1=Alu.add,
                )
            for half in range(2):
                s = half * (NCH * N // 2)
                t = s + NCH * N // 2
                nc.vector.scalar_tensor_tensor(
                    out=e2[:, s:t], in0=e2[:, s:t], scalar=ALPHA,
                    in1=e2[:, s:t], op0=Alu.mult, op1=Alu.max,
                )
            nc.scalar.activation(e2[:, :], e2[:, :], Act.Exp)

            for ic in range(NCH):
                po = pmm.tile([128, C + 1], dt.float32, tag="mm")
                for jc in range(NCH):
                    nc.tensor.matmul(
                        po[:, :],
                        e2[:, jc * N + ic * 128 : jc * N + ic * 128 + 128],
                        gx[:, jc * 260 : jc * 260 + C + 1],
                        start=(jc == 0), stop=(jc == NCH - 1),
                    )
                rz2 = sm.tile([128, 1], dt.float32, tag="rz2")
                nc.vector.reciprocal(rz2[:, :], po[:, C : C + 1])
                y = sm.tile([128, C], dt.bfloat16, tag="y")
                nc.vector.tensor_scalar(
                    out=y[:, :], in0=po[:, 0:C], scalar1=rz2[:, :], scalar2=None,
                    op0=Alu.mult,
                )
                e3 = sm.tile([128, C], dt.bfloat16, tag="e3")
                nc.scalar.activation(e3[:, :], y[:, :], Act.Exp)
                r2 = sm.tile([128, C], dt.bfloat16, tag="r2")
                nc.vector.tensor_scalar(
                    out=r2[:, :], in0=e3[:, :], scalar1=-1.0, scalar2=0.0,
                    op0=Alu.add, op1=Alu.min,
                )
                el = sm.tile([128, C], dt.bfloat16, tag="el")
                nc.vector.scalar_tensor_tensor(
                    out=el[:, :], in0=y[:, :], scalar=0.0, in1=r2[:, :],
                    op0=Alu.max, op1=Alu.add,
                )
                ofin = sm.tile([128, C], dt.float32, tag="ofin")
                nc.vector.tensor_tensor(
                    out=ofin[:, :], in0=el[:, :],
                    in1=xs_sb[:, ic * F : (ic + 1) * F], op=Alu.add,
                )
                nc.sync.dma_start(
                    out=out_d[ic * 128 : (ic + 1) * 128, :], in_=ofin[:, :]
                )

    nc.compile()
    return nc


def get_program():
    if "nc" not in _CACHE:
        _CACHE["nc"] = _build_program()
    return _CACHE["nc"]


def make_in_maps(x, adj, W, Wb, a, ab, Wo, Wob, ao, aob):
    x = np.asarray(x, np.float32)
    adj = np.asarray(adj)
    W = np.asarray(W, np.float32)
    Wb = np.asarray(Wb, np.float32)
    a = np.asarray(a, np.float32)
    ab = np.asarray(ab, np.float32)
    Wo = np.asarray(Wo, np.float32)
    Wob = np.asarray(Wob, np.float32)
    ao = np.asarray(ao, np.float32)
    aob = np.asarray(aob, np.float32)

    # W_all[f, h*D+d] = W[h, f, d];  Wb row flattened the same way
    W_all = W.transpose(1, 0, 2).reshape(F, HD)
    wb_row = Wb.reshape(1, HD)
    wp = np.concatenate([W_all, wb_row], axis=0).astype(BF16)  # [257, 512]

    # V_l[f,h] = sum_d W[h,f,d] a[h,d];  consts fold Wb and ab
    V_l = np.einsum("hfd,hd->fh", W, a[:, :D]).astype(np.float32)
    V_r = np.einsum("hfd,hd->fh", W, a[:, D:]).astype(np.float32)
    const_l = (Wb * a[:, :D]).sum(1) + ab  # [H]
    const_r = (Wb * a[:, D:]).sum(1)
    vlr = np.concatenate(
        [
            np.concatenate([V_l, V_r], axis=1),
            np.concatenate([const_l, const_r])[None, :],
        ],
        axis=0,
    ).astype(BF16)  # [257, 16]

    u_l = Wo @ ao[:C]  # [512]
    u_r = Wo @ ao[C:]
    wo_top = np.concatenate([Wo, u_l[:, None], u_r[:, None]], axis=1)  # [512, 258]
    wo_bot = np.concatenate(
        [Wob, [Wob @ ao[:C] + aob], [Wob @ ao[C:]]]
    )[None, :]  # [1, 258]
    wo_ext = np.concatenate([wo_top, wo_bot], axis=0).astype(BF16)  # [513, 258]

    ones_row = np.ones((1, N), BF16)
    in_maps = []
    for b in range(B):
        xt = np.concatenate([x[b].T.astype(BF16), ones_row], axis=0)  # [257, 1024]
        lmtb = np.where(adj[b].T > 0, np.float32(0.0), np.float32(NEGM)).astype(BF16)
        in_maps.append(
            {
                "xt": np.ascontiguousarray(xt),
                "xs": np.ascontiguousarray(x[b]),
                "lmt": np.ascontiguousarray(lmtb),
                "wp": wp,
                "vlr": vlr,
                "wo": wo_ext,
            }
        )
    return in_maps


def kernel(**inputs) -> np.ndarray:
    from concourse.bass_utils import run_bass_kernel_spmd

    nc = get_program()
    in_maps = make_in_maps(**inputs)
    res = run_bass_kernel_spmd(nc, in_maps, core_ids=list(range(B)))
    return np.stack([res.results[b]["out"] for b in range(B)], axis=0)


# revision 18
# speedup vs baseline: 1.2239x; 1.2239x over previous
"""Trainium2 Bass kernel for a 2-layer GAT (B=8, N=1024, F=256, D=64, H=8, C=256).

Sharding: data-parallel over batch — one batch element per NeuronCore (8 cores).

Per-core algorithm (all layouts chosen so softmax needs no transposes):
  h      = x @ W_all (+Wb)                          [n, 512]   PE, bf16
  sl/sr  = x @ V_l / V_r (+consts, ab folded)       [n, 16]    PE (same lhsT)
  scoresT[j,i] = LR(sl_i + sr_j + ab) + mask        [j, i]     built directly
      S1: x = (sl_bcast + sr_j) + logmT   one scalar_tensor_tensor per j-chunk
      S2: u = (x*0.2) max x               one scalar_tensor_tensor (LeakyReLU)
      S3: e = Exp(u)                      one ACT sweep
      (mask folded additively pre-LR as -16384; exp underflows to exact 0)
  agg:   out[i, 65h] = sum_j e[j,i] * [h_h | 1]     PE; ones col gives Z_i
  hh     = num / Z ; z = ELU(hh) = relu(hh) + min(exp(hh)-1, 0)
  layer 2 identical with g = z @ Wo (+u_l/u_r cols for tl/tr), C=256
  out    = ELU(a2 @ g / Z2) + x
"""

import numpy as np
import ml_dtypes
from contextlib import ExitStack

BF16 = ml_dtypes.bfloat16
B, N, F, D, H, C = 8, 1024, 256, 64, 8, 256
HD = H * D  # 512
NEGM = -16384.0  # mask offset; LR then exp underflows to exact 0
ALPHA = 0.2

_CACHE = {}


def _build_program():
    import concourse.bacc as bacc
    import concourse.bass as bass
    import concourse.mybir as mybir
    from concourse.tile import TileContext
    from concourse.masks import make_identity

    dt = mybir.dt
    Alu = mybir.AluOpType
    Act = mybir.ActivationFunctionType

    nc = bacc.Bacc()

    xt = nc.declare_dram_parameter("xt", [F + 1, N], dt.bfloat16, isOutput=False)
    xs = nc.declare_dram_parameter("xs", [N, F], dt.float32, isOutput=False)
    msk = nc.declare_dram_parameter("msk", [N, N], dt.bfloat16, isOutput=False)
    wp = nc.declare_dram_parameter("wp", [F + 1, HD], dt.bfloat16, isOutput=False)
    vlr = nc.declare_dram_parameter("vlr", [F + 1, 2 * H], dt.bfloat16, isOutput=False)
    wo = nc.declare_dram_parameter("wo", [HD + 1, C + 2], dt.bfloat16, isOutput=False)
    out_d = nc.declare_dram_parameter("out", [N, C], dt.float32, isOutput=True)

    rows_d = nc.dram_tensor("rows_bounce", [2 * H + 2, N], dt.bfloat16)

    NCH = N // 128  # 8 chunks of 128 nodes

    def bcast128(row_ap):
        # [1, N] DRAM row -> [128, N] partition-broadcast read for DMA
        return bass.AP(
            tensor=row_ap.tensor,
            offset=row_ap.offset,
            ap=[[0, 128]] + list(row_ap.ap),
        )

    with TileContext(nc) as tc:
        with ExitStack() as ctx:
            cons = ctx.enter_context(tc.tile_pool(name="cons", bufs=1))
            bc = ctx.enter_context(tc.tile_pool(name="bc", bufs=3))
            eb = ctx.enter_context(tc.tile_pool(name="eb", bufs=5))
            tb = ctx.enter_context(tc.tile_pool(name="tb", bufs=2))
            wk = ctx.enter_context(tc.tile_pool(name="wk", bufs=3))
            sm = ctx.enter_context(tc.tile_pool(name="sm", bufs=2))
            pmm = ctx.enter_context(tc.tile_pool(name="pmm", bufs=3, space="PSUM"))
            pm2 = ctx.enter_context(tc.tile_pool(name="pm2", bufs=2, space="PSUM"))
            ptp = ctx.enter_context(tc.tile_pool(name="ptp", bufs=1, space="PSUM"))

            # ---------- constants / params ----------
            ident_f = cons.tile([128, 128], dt.float32)
            make_identity(nc, ident_f[:, :])
            ident_b = cons.tile([128, 128], dt.bfloat16)
            make_identity(nc, ident_b[:, :])

            xt_sb = cons.tile([128, 2 * N], dt.bfloat16)
            nc.sync.dma_start(out=xt_sb[:, 0:N], in_=xt[0:128, :])
            nc.sync.dma_start(out=xt_sb[:, N : 2 * N], in_=xt[128:256, :])
            xt_one = cons.tile([1, N], dt.bfloat16)
            nc.sync.dma_start(out=xt_one[:, :], in_=xt[256:257, :])

            wp_sb = cons.tile([128, 2 * HD], dt.bfloat16)
            nc.sync.dma_start(out=wp_sb[:, 0:HD], in_=wp[0:128, :])
            nc.sync.dma_start(out=wp_sb[:, HD : 2 * HD], in_=wp[128:256, :])
            wp_one = cons.tile([1, HD], dt.bfloat16)
            nc.sync.dma_start(out=wp_one[:, :], in_=wp[256:257, :])

            vlr_sb = cons.tile([128, 4 * H], dt.bfloat16)
            nc.sync.dma_start(out=vlr_sb[:, 0 : 2 * H], in_=vlr[0:128, :])
            nc.sync.dma_start(out=vlr_sb[:, 2 * H : 4 * H], in_=vlr[128:256, :])
            vlr_one = cons.tile([1, 2 * H], dt.bfloat16)
            nc.sync.dma_start(out=vlr_one[:, :], in_=vlr[256:257, :])

            wo_sb = cons.tile([128, 4 * (C + 2)], dt.bfloat16)
            for k in range(4):
                nc.sync.dma_start(
                    out=wo_sb[:, k * (C + 2) : (k + 1) * (C + 2)],
                    in_=wo[k * 128 : (k + 1) * 128, :],
                )
            wo_one = cons.tile([1, C + 2], dt.bfloat16)
            nc.sync.dma_start(out=wo_one[:, :], in_=wo[HD : HD + 1, :])

            msk_sb = cons.tile([128, NCH * N], dt.bfloat16)
            for c in range(NCH):
                nc.sync.dma_start(
                    out=msk_sb[:, c * N : (c + 1) * N],
                    in_=msk[c * 128 : (c + 1) * 128, :],
                )

            # ---------- phase 1: h = x@W_all, sl/sr = x@VLR ----------
            hx = cons.tile([128, NCH * 8 * 66], dt.bfloat16)  # [h(64)|1|pad] per head
            nc.vector.memset(
                hx[:, :].rearrange("p (n h s) -> p n h s", h=8, s=66)[:, :, :, 64:65],
                1.0,
            )
            slsr = cons.tile([128, NCH * 16], dt.float32)

            for n in range(NCH):
                ph = pmm.tile([128, HD], dt.float32, tag="mm")
                ps = pm2.tile([128, 16], dt.float32, tag="mm2")
                for k in range(2):
                    lt = xt_sb[:, k * N + n * 128 : k * N + n * 128 + 128]
                    nc.tensor.matmul(
                        ph[:, :], lt, wp_sb[:, k * HD : (k + 1) * HD],
                        start=(k == 0), stop=False,
                    )
                    nc.tensor.matmul(
                        ps[:, :], lt, vlr_sb[:, k * 16 : (k + 1) * 16],
                        start=(k == 0), stop=False,
                    )
                lt1 = xt_one[:, n * 128 : n * 128 + 128]
                nc.tensor.matmul(ph[:, :], lt1, wp_one[:, :], start=False, stop=True)
                nc.tensor.matmul(ps[:, :], lt1, vlr_one[:, :], start=False, stop=True)

                # exit h -> hx (bf16, 66-stride blocks; ones cols pre-set)
                hx_v = hx[:, n * 528 : (n + 1) * 528].rearrange(
                    "p (h s) -> p h s", s=66
                )[:, :, 0:64]
                ph_v = ph[:, :].rearrange("p (h s) -> p h s", s=64)
                nc.scalar.activation(hx_v, ph_v, Act.Copy)
                nc.vector.tensor_copy(out=slsr[:, n * 16 : (n + 1) * 16], in_=ps[:, :])

            # ---------- phase 1b: transpose sl/sr to rows; bounce via DRAM ----
            pt = ptp.tile([16, N], dt.float32, tag="tp")
            for n in range(NCH):
                nc.tensor.transpose(
                    pt[:, n * 128 : (n + 1) * 128],
                    slsr[:, n * 16 : (n + 1) * 16],
                    ident_f[:, :],
                )
            slsrT = cons.tile([16, N], dt.bfloat16)
            nc.scalar.activation(slsrT[:, :], pt[:, :], Act.Copy)
            nc.sync.dma_start(out=rows_d[0:16, :], in_=slsrT[:, :])

            # ---------- phase 2: attention per head, quads for agg exits ------
            zacc = cons.tile([128, NCH * 528], dt.float32)
            z_sb = cons.tile([128, NCH * HD], dt.bfloat16)

            HN = NCH * N // 2  # half-sweep width

            def score_sweep(slb, sr_col, act_path):
                """Builds masked exp'd scores [j, i] for one head into a tile.

                slb: [128, N] broadcast of the free-dim term (sl or tl)
                sr_col(c): [128, 1] per-partition scalar AP for j-chunk c
                Returns the e tile [128, NCH*N] (masked exp'd scores).
                """
                e = eb.tile([128, NCH * N], dt.bfloat16, tag="e")
                if act_path:
                    # ACT computes LeakyReLU(slb + sr) directly (bias trick)
                    for c in range(NCH):
                        nc.scalar.activation(
                            e[:, c * N : (c + 1) * N], slb[:, :], Act.Lrelu,
                            bias=sr_col(c), scale=1.0, alpha=ALPHA,
                        )
                else:
                    for half in range(2):
                        s = half * HN
                        for ch in range(NCH // 2):
                            c = half * (NCH // 2) + ch
                            nc.vector.tensor_scalar(
                                out=e[:, c * N : (c + 1) * N], in0=slb[:, :],
                                scalar1=sr_col(c), scalar2=None, op0=Alu.add,
                            )
                        t = tb.tile([128, HN], dt.bfloat16, tag="t")
                        nc.vector.tensor_scalar(
                            out=t[:, :], in0=e[:, s : s + HN], scalar1=ALPHA,
                            scalar2=None, op0=Alu.mult,
                        )
                        nc.vector.tensor_tensor(
                            out=e[:, s : s + HN], in0=t[:, :],
                            in1=e[:, s : s + HN], op=Alu.max,
                        )
                nc.scalar.activation(e[:, :], e[:, :], Act.Exp)
                # mask multiplicatively (exact: exp of masked ref is 0)
                nc.vector.tensor_tensor(
                    out=e[:, :], in0=e[:, :], in1=msk_sb[:, :], op=Alu.mult
                )
                return e

            ACT_HEADS = (0, 2, 4, 6)
            for q in range(2):
                ebufs = []
                for hq in range(4):
                    h = 4 * q + hq
                    slb = bc.tile([128, N], dt.bfloat16, tag="slb")
                    nc.sync.dma_start(out=slb[:, :], in_=bcast128(rows_d[h : h + 1, :]))
                    em = score_sweep(
                        slb,
                        lambda c, h=h: slsr[:, c * 16 + 8 + h : c * 16 + 9 + h],
                        act_path=(h in ACT_HEADS),
                    )
                    ebufs.append(em)

                for ic in range(NCH):
                    pa = pmm.tile([128, 260], dt.float32, tag="mm")
                    for hq in range(4):
                        h = 4 * q + hq
                        e = ebufs[hq]
                        for jc in range(NCH):
                            nc.tensor.matmul(
                                pa[:, hq * 65 : hq * 65 + 65],
                                e[:, jc * N + ic * 128 : jc * N + ic * 128 + 128],
                                hx[:, jc * 528 + h * 66 : jc * 528 + h * 66 + 65],
                                start=(jc == 0),
                                stop=(jc == NCH - 1),
                            )
                    nc.scalar.activation(
                        zacc[:, ic * 528 + q * 264 : ic * 528 + q * 264 + 260],
                        pa[:, :],
                        Act.Copy,
                    )

            # ---------- phase 2b: divide by Z, ELU -> z ----------
            rz_all = cons.tile([128, NCH * 8], dt.float32)
            for ic in range(NCH):
                rzrep = wk.tile([128, HD], dt.float32, tag="rzrep")
                hh = wk.tile([128, HD], dt.bfloat16, tag="hh")
                for q in range(2):
                    zq = zacc[:, ic * 528 + q * 264 : ic * 528 + q * 264 + 260]
                    zq_v = zq.rearrange("p (h s) -> p h s", s=65)
                    rz = rz_all[:, ic * 8 + q * 4 : ic * 8 + q * 4 + 4]
                    nc.vector.reciprocal(
                        rz[:, :].rearrange("p (h s) -> p h s", s=1),
                        zq_v[:, :, 64:65],
                    )
                    nc.vector.tensor_copy(
                        out=rzrep[:, q * 256 : (q + 1) * 256].rearrange(
                            "p (h s) -> p h s", s=64
                        ),
                        in_=rz[:, :]
                        .rearrange("p (h s) -> p h s", s=1)
                        .to_broadcast([128, 4, 64]),
                    )
                    nc.vector.tensor_tensor(
                        out=hh[:, q * 256 : (q + 1) * 256].rearrange(
                            "p (h s) -> p h s", s=64
                        ),
                        in0=zq_v[:, :, 0:64],
                        in1=rzrep[:, q * 256 : (q + 1) * 256].rearrange(
                            "p (h s) -> p h s", s=64
                        ),
                        op=Alu.mult,
                    )
                ee = wk.tile([128, HD], dt.bfloat16, tag="ee")
                nc.scalar.activation(ee[:, :], hh[:, :], Act.Exp)
                r1 = wk.tile([128, HD], dt.bfloat16, tag="r1")
                nc.vector.tensor_scalar(
                    out=r1[:, :], in0=ee[:, :], scalar1=-1.0, scalar2=0.0,
                    op0=Alu.add, op1=Alu.min,
                )
                nc.vector.scalar_tensor_tensor(
                    out=z_sb[:, ic * HD : (ic + 1) * HD],
                    in0=hh[:, :], scalar=0.0, in1=r1[:, :],
                    op0=Alu.max, op1=Alu.add,
                )

            # ---------- phase 3: zT via PE transpose ----------
            zt_sb = cons.tile([128, 4 * N], dt.bfloat16)
            zt_one = cons.tile([1, N], dt.bfloat16)
            nc.vector.memset(zt_one[:, :], 1.0)
            for kc in range(4):
                pz = pmm.tile([128, N], dt.bfloat16, tag="mm")
                for ic in range(NCH):
                    nc.tensor.transpose(
                        pz[:, ic * 128 : (ic + 1) * 128],
                        z_sb[:, ic * HD + kc * 128 : ic * HD + kc * 128 + 128],
                        ident_b[:, :],
                    )
                nc.vector.tensor_copy(out=zt_sb[:, kc * N : (kc + 1) * N], in_=pz[:, :])

            # ---------- phase 4: g = z@Wo (+tl/tr cols) ----------
            gx = cons.tile([128, NCH * 260], dt.bfloat16)
            nc.vector.memset(
                gx[:, :].rearrange("p (n s) -> p n s", s=260)[:, :, 256:257], 1.0
            )
            glgr = cons.tile([128, NCH * 2], dt.float32)
            for n in range(NCH):
                pg = pmm.tile([128, C + 2], dt.float32, tag="mm")
                for kc in range(4):
                    nc.tensor.matmul(
                        pg[:, :],
                        zt_sb[:, kc * N + n * 128 : kc * N + n * 128 + 128],
                        wo_sb[:, kc * (C + 2) : (kc + 1) * (C + 2)],
                        start=(kc == 0), stop=False,
                    )
                nc.tensor.matmul(
                    pg[:, :], zt_one[:, n * 128 : n * 128 + 128], wo_one[:, :],
                    start=False, stop=True,
                )
                nc.scalar.activation(
                    gx[:, n * 260 : n * 260 + C], pg[:, 0:C], Act.Copy
                )
                nc.vector.tensor_copy(
                    out=glgr[:, n * 2 : (n + 1) * 2], in_=pg[:, C : C + 2]
                )

            # ---------- phase 4b: gl/gr rows ----------
            pt2 = ptp.tile([2, N], dt.float32, tag="tp")
            for n in range(NCH):
                nc.tensor.transpose(
                    pt2[:, n * 128 : (n + 1) * 128],
                    glgr[:, n * 2 : (n + 1) * 2],
                    ident_f[:, :],
                )
            ggT = cons.tile([2, N], dt.bfloat16)
            nc.scalar.activation(ggT[:, :], pt2[:, :], Act.Copy)
            nc.sync.dma_start(out=rows_d[16:18, :], in_=ggT[:, :])

            # ---------- phase 5: output attention layer ----------
            glb = bc.tile([128, N], dt.bfloat16, tag="slb")
            nc.sync.dma_start(out=glb[:, :], in_=bcast128(rows_d[16:17, :]))
            e2 = score_sweep(
                glb, lambda c: glgr[:, c * 2 + 1 : c * 2 + 2], act_path=False
            )

            for ic in range(NCH):
                po = pmm.tile([128, C + 1], dt.float32, tag="mm")
                for jc in range(NCH):
                    nc.tensor.matmul(
                        po[:, :],
                        e2[:, jc * N + ic * 128 : jc * N + ic * 128 + 128],
                        gx[:, jc * 260 : jc * 260 + C + 1],
                        start=(jc == 0), stop=(jc == NCH - 1),
                    )
                rz2 = sm.tile([128, 1], dt.float32, tag="rz2")
                nc.vector.reciprocal(rz2[:, :], po[:, C : C + 1])
                y = sm.tile([128, C], dt.bfloat16, tag="y")
                nc.vector.tensor_scalar(
                    out=y[:, :], in0=po[:, 0:C], scalar1=rz2[:, :], scalar2=None,
                    op0=Alu.mult,
                )
                e3 = sm.tile([128, C], dt.bfloat16, tag="e3")
                nc.scalar.activation(e3[:, :], y[:, :], Act.Exp)
                r2 = sm.tile([128, C], dt.bfloat16, tag="r2")
                nc.vector.tensor_scalar(
                    out=r2[:, :], in0=e3[:, :], scalar1=-1.0, scalar2=0.0,
                    op0=Alu.add, op1=Alu.min,
                )
                el = sm.tile([128, C], dt.bfloat16, tag="el")
                nc.vector.scalar_tensor_tensor(
                    out=el[:, :], in0=y[:, :], scalar=0.0, in1=r2[:, :],
                    op0=Alu.max, op1=Alu.add,
                )
                xs5 = sm.tile([128, F], dt.float32, tag="xs5")
                nc.sync.dma_start(
                    out=xs5[:, :], in_=xs[ic * 128 : (ic + 1) * 128, :]
                )
                ofin = sm.tile([128, C], dt.float32, tag="ofin")
                nc.vector.tensor_tensor(
                    out=ofin[:, :], in0=el[:, :], in1=xs5[:, :], op=Alu.add,
                )
                nc.sync.dma_start(
                    out=out_d[ic * 128 : (ic + 1) * 128, :], in_=ofin[:, :]
                )

    nc.compile()
    return nc


def get_program():
    if "nc" not in _CACHE:
        _CACHE["nc"] = _build_program()
    return _CACHE["nc"]


def make_in_maps(x, adj, W, Wb, a, ab, Wo, Wob, ao, aob):
    x = np.asarray(x, np.float32)
    adj = np.asarray(adj)
    W = np.asarray(W, np.float32)
    Wb = np.asarray(Wb, np.float32)
    a = np.asarray(a, np.float32)
    ab = np.asarray(ab, np.float32)
    Wo = np.asarray(Wo, np.float32)
    Wob = np.asarray(Wob, np.float32)
    ao = np.asarray(ao, np.float32)
    aob = np.asarray(aob, np.float32)

    # W_all[f, h*D+d] = W[h, f, d];  Wb row flattened the same way
    W_all = W.transpose(1, 0, 2).reshape(F, HD)
    wb_row = Wb.reshape(1, HD)
    wp = np.concatenate([W_all, wb_row], axis=0).astype(BF16)  # [257, 512]

    # V_l[f,h] = sum_d W[h,f,d] a[h,d];  consts fold Wb and ab
    V_l = np.einsum("hfd,hd->fh", W, a[:, :D]).astype(np.float32)
    V_r = np.einsum("hfd,hd->fh", W, a[:, D:]).astype(np.float32)
    const_l = (Wb * a[:, :D]).sum(1) + ab  # [H]
    const_r = (Wb * a[:, D:]).sum(1)
    vlr = np.concatenate(
        [
            np.concatenate([V_l, V_r], axis=1),
            np.concatenate([const_l, const_r])[None, :],
        ],
        axis=0,
    ).astype(BF16)  # [257, 16]

    u_l = Wo @ ao[:C]  # [512]
    u_r = Wo @ ao[C:]
    wo_top = np.concatenate([Wo, u_l[:, None], u_r[:, None]], axis=1)  # [512, 258]
    wo_bot = np.concatenate(
        [Wob, [Wob @ ao[:C] + aob], [Wob @ ao[C:]]]
    )[None, :]  # [1, 258]
    wo_ext = np.concatenate([wo_top, wo_bot], axis=0).astype(BF16)  # [513, 258]

    ones_row = np.ones((1, N), BF16)
    in_maps = []
    for b in range(B):
        xt = np.concatenate([x[b].T.astype(BF16), ones_row], axis=0)  # [257, 1024]
        mb = np.where(adj[b].T > 0, np.float32(1.0), np.float32(0.0)).astype(BF16)
        in_maps.append(
            {
                "xt": np.ascontiguousarray(xt),
                "xs": np.ascontiguousarray(x[b]),
                "msk": np.ascontiguousarray(mb),
                "wp": wp,
                "vlr": vlr,
                "wo": wo_ext,
            }
        )
    return in_maps


def kernel(**inputs) -> np.ndarray:
    from concourse.bass_utils import run_bass_kernel_spmd

    nc = get_program()
    in_maps = make_in_maps(**inputs)
    res = run_bass_kernel_spmd(nc, in_maps, core_ids=list(range(B)))
    return np.stack([res.results[b]["out"] for b in range(B)], axis=0)
